# revision 1
# baseline (speedup 1.0000x reference)
"""Cross-modal attention Trainium2 kernel (Bass/Tile), data-parallel over batch.

Per core (one batch element):
    q = img @ Wq.T + bq ; k = ts @ Wk.T + bk ; v = ts @ Wv.T + bv
    out = softmax(q @ k.T) @ v

Layout strategy (contraction dim must live on SBUF partitions):
  - img/ts are PE-transposed tile-wise into imgT/tsT [d, n].
  - Weights are PE-transposed into WT [d, e] (fp32r).
  - qT/kT computed as [e, n] (e on partitions) directly; v as [j, e] natural.
  - scores tile S[i=128, j=2048] accumulated in PSUM (4 banks), row-softmax via
    DVE reduce_max(negate) + ACT Exp(bias=-max, accum_out=row sums).
  - probs PE-transposed per 128-col block into probsT [j, i] for the PV matmul;
    1/sum applied to the PV result (cheaper than scaling probs).
  - All big matmuls use fp32r (11-bit mantissa, 4x faster than fp32 on PE).
"""

import numpy as np

import concourse.bass as bass
import concourse.mybir as mybir
import concourse.tile as tile
from concourse import bacc
from concourse.bass_utils import run_bass_kernel_spmd
from concourse.masks import make_identity

B, NQ, NK, D = 8, 2048, 2048, 512
P = 128
DC = D // P        # 4 contraction chunks
EC = D // P        # 4 output-dim chunks
TQ = NQ // P       # 16 query tiles
TK = NK // P       # 16 key tiles
JC = NK // 512     # 4 key chunks of 512 (scores free dim)
IC = NQ // 512     # 4 query chunks of 512 (projection free dim)

F32 = mybir.dt.float32
F32R = mybir.dt.float32r
AX = mybir.AxisListType.X


def build():
    nc = bacc.Bacc(None, target_bir_lowering=False)

    img = nc.dram_tensor("img", [NQ, D], F32, kind="ExternalInput")
    ts = nc.dram_tensor("ts", [NK, D], F32, kind="ExternalInput")
    Wq = nc.dram_tensor("Wq", [D, D], F32, kind="ExternalInput")
    Wk = nc.dram_tensor("Wk", [D, D], F32, kind="ExternalInput")
    Wv = nc.dram_tensor("Wv", [D, D], F32, kind="ExternalInput")
    bq = nc.dram_tensor("bq", [D], F32, kind="ExternalInput")
    bk = nc.dram_tensor("bk", [D], F32, kind="ExternalInput")
    bv = nc.dram_tensor("bv", [D], F32, kind="ExternalInput")
    out = nc.dram_tensor("out", [NQ, D], F32, kind="ExternalOutput")

    with tile.TileContext(nc) as tc:
        with (
            tc.tile_pool(name="const", bufs=1) as const_pool,
            tc.tile_pool(name="big", bufs=1) as big,
        ):
            ident = const_pool.tile([P, P], F32)
            make_identity(nc, ident)
            ident_r = const_pool.tile([P, P], F32R)
            nc.vector.tensor_copy(ident_r[:], ident[:])

            # biases: bq/bk as [P, EC] (e%128 on partitions), bv replicated [P, D]
            bq_sb = const_pool.tile([P, EC], F32)
            nc.sync.dma_start(bq_sb[:], bq.ap().rearrange("(c p) -> p c", p=P))
            bk_sb = const_pool.tile([P, EC], F32)
            nc.sync.dma_start(bk_sb[:], bk.ap().rearrange("(c p) -> p c", p=P))
            bv_sb = const_pool.tile([P, D], F32)
            nc.gpsimd.dma_start(bv_sb[:], bv.ap().partition_broadcast(P))

            # persistent big operands (fp32r)
            qT = big.tile([P, EC, NQ], F32R)   # qT[p, ec, i] = q[i, ec*128+p]
            kT = big.tile([P, EC, NK], F32R)
            v_sb = big.tile([P, TK, D], F32R)  # v_sb[p, jt, e] = v[jt*128+p, e]

            # ---- Phase 0: weight transposes WT[d, e] (fp32r) ----
            with (
                tc.tile_pool(name="wstage", bufs=2) as wstage,
                tc.tile_pool(name="wps", bufs=4, space="PSUM") as wps,
                tc.tile_pool(name="wt", bufs=1) as wtp,
            ):
                WTs = {}
                for wname, wdram in (("q", Wq), ("k", Wk), ("v", Wv)):
                    w_sb = wstage.tile([P, EC, D], F32, tag="wstage")
                    nc.sync.dma_start(
                        w_sb[:], wdram.ap().rearrange("(ec p) d -> p ec d", p=P)
                    )
                    WT = wtp.tile([P, DC, D], F32R, tag=f"wt_{wname}")
                    for ec in range(EC):
                        for dc in range(DC):
                            pw = wps.tile([P, P], F32, tag="wps")
                            nc.tensor.transpose(
                                pw[:], w_sb[:, ec, dc * P : (dc + 1) * P], ident[:]
                            )
                            nc.vector.tensor_copy(
                                WT[:, dc, ec * P : (ec + 1) * P], pw[:]
                            )
                    WTs[wname] = WT

                # ---- Phase 1+2: imgT, then qT ----
                with tc.tile_pool(name="xt", bufs=1) as xtp:
                    imgT = xtp.tile([P, DC, NQ], F32R, tag="xT")
                    with (
                        tc.tile_pool(name="xstage", bufs=3) as xstage,
                        tc.tile_pool(name="xps", bufs=4, space="PSUM") as xps,
                    ):
                        for it in range(TQ):
                            x_sb = xstage.tile([P, D], F32, tag="xstage")
                            nc.sync.dma_start(x_sb[:], img[it * P : (it + 1) * P, :])
                            for dc in range(DC):
                                px = xps.tile([P, P], F32, tag="xps")
                                nc.tensor.transpose(
                                    px[:], x_sb[:, dc * P : (dc + 1) * P], ident[:]
                                )
                                nc.vector.tensor_copy(
                                    imgT[:, dc, it * P : (it + 1) * P], px[:]
                                )
                    with tc.tile_pool(name="pps", bufs=4, space="PSUM") as pps:
                        for ic in range(IC):
                            for ec in range(EC):
                                pq = pps.tile([P, 512], F32, tag="pps")
                                for dc in range(DC):
                                    nc.tensor.matmul(
                                        pq[:],
                                        WTs["q"][:, dc, ec * P : (ec + 1) * P],
                                        imgT[:, dc, ic * 512 : (ic + 1) * 512],
                                        start=(dc == 0),
                                        stop=(dc == DC - 1),
                                    )
                                nc.vector.tensor_scalar_add(
                                    qT[:, ec, ic * 512 : (ic + 1) * 512],
                                    pq[:],
                                    bq_sb[:, ec : ec + 1],
                                )

                # ---- Phase 3: tsT, then kT and v ----
                with tc.tile_pool(name="xt2", bufs=1) as xtp2:
                    tsT = xtp2.tile([P, DC, NK], F32R, tag="xT2")
                    with (
                        tc.tile_pool(name="xstage2", bufs=3) as xstage2,
                        tc.tile_pool(name="xps2", bufs=4, space="PSUM") as xps2,
                    ):
                        for it in range(TK):
                            x_sb = xstage2.tile([P, D], F32, tag="xstage2")
                            nc.sync.dma_start(x_sb[:], ts[it * P : (it + 1) * P, :])
                            for dc in range(DC):
                                px = xps2.tile([P, P], F32, tag="xps2")
                                nc.tensor.transpose(
                                    px[:], x_sb[:, dc * P : (dc + 1) * P], ident[:]
                                )
                                nc.vector.tensor_copy(
                                    tsT[:, dc, it * P : (it + 1) * P], px[:]
                                )
                    with tc.tile_pool(name="pps2", bufs=4, space="PSUM") as pps2:
                        for ic in range(IC):
                            for ec in range(EC):
                                pk = pps2.tile([P, 512], F32, tag="pps2")
                                for dc in range(DC):
                                    nc.tensor.matmul(
                                        pk[:],
                                        WTs["k"][:, dc, ec * P : (ec + 1) * P],
                                        tsT[:, dc, ic * 512 : (ic + 1) * 512],
                                        start=(dc == 0),
                                        stop=(dc == DC - 1),
                                    )
                                nc.vector.tensor_scalar_add(
                                    kT[:, ec, ic * 512 : (ic + 1) * 512],
                                    pk[:],
                                    bk_sb[:, ec : ec + 1],
                                )
                        for jt in range(TK):
                            pv = pps2.tile([P, 512], F32, tag="pps2")
                            for dc in range(DC):
                                nc.tensor.matmul(
                                    pv[:],
                                    tsT[:, dc, jt * P : (jt + 1) * P],
                                    WTs["v"][:, dc, :],
                                    start=(dc == 0),
                                    stop=(dc == DC - 1),
                                )
                            nc.vector.tensor_add(v_sb[:, jt, :], pv[:], bv_sb[:])

            # ---- Phase 4: attention ----
            with (
                tc.tile_pool(name="sps", bufs=1, space="PSUM") as sps,
                tc.tile_pool(name="tps", bufs=2, space="PSUM") as tps,
                tc.tile_pool(name="ops", bufs=2, space="PSUM") as ops,
                tc.tile_pool(name="soft", bufs=2) as soft,
                tc.tile_pool(name="outp", bufs=2) as outp,
            ):
                for qt in range(TQ):
                    S = sps.tile([P, NK], F32, tag="S")
                    for jc in range(JC):
                        for ec in range(EC):
                            nc.tensor.matmul(
                                S[:, jc * 512 : (jc + 1) * 512],
                                qT[:, ec, qt * P : (qt + 1) * P],
                                kT[:, ec, jc * 512 : (jc + 1) * 512],
                                start=(ec == 0),
                                stop=(ec == EC - 1),
                            )
                    negmax = soft.tile([P, 1], F32, tag="negmax")
                    nc.vector.reduce_max(negmax[:], S[:], axis=AX, negate=True)
                    probs = soft.tile([P, NK], F32R, tag="probs")
                    rowsum = soft.tile([P, 1], F32, tag="rowsum")
                    nc.scalar.activation(
                        out=probs[:],
                        in_=S[:],
                        func=mybir.ActivationFunctionType.Exp,
                        bias=negmax[:],
                        scale=1.0,
                        accum_out=rowsum[:],
                    )
                    recip = soft.tile([P, 1], F32, tag="recip")
                    nc.vector.reciprocal(recip[:], rowsum[:])

                    probsT = soft.tile([P, TK, P], F32R, tag="probsT")
                    for g in range(TK // 4):
                        pt = tps.tile([P, 512], F32R, tag="pt")
                        for u in range(4):
                            jt = 4 * g + u
                            nc.tensor.transpose(
                                pt[:, u * P : (u + 1) * P],
                                probs[:, jt * P : (jt + 1) * P],
                                ident_r[:],
                                # 4 independent transposes share one PSUM bank
                                # (disjoint 128-col quarters)
                            )
                        nc.vector.tensor_copy(
                            probsT[:, 4 * g : 4 * g + 4, :], pt[:]
                        )

                    po = ops.tile([P, D], F32, tag="po")
                    for jt in range(TK):
                        nc.tensor.matmul(
                            po[:],
                            probsT[:, jt, :],
                            v_sb[:, jt, :],
                            start=(jt == 0),
                            stop=(jt == TK - 1),
                        )
                    o_sb = outp.tile([P, D], F32, tag="o")
                    nc.vector.tensor_scalar_mul(o_sb[:], po[:], recip[:])
                    nc.sync.dma_start(out[qt * P : (qt + 1) * P, :], o_sb[:])

    nc.compile()
    return nc


_NC_CACHE = None


def _get_nc():
    global _NC_CACHE
    if _NC_CACHE is None:
        _NC_CACHE = build()
    return _NC_CACHE


def run(inputs: dict, trace: bool = False):
    """Run on 8 cores, batch-parallel. Returns (out [B,NQ,D], BassKernelResults)."""
    nc = _get_nc()
    in_maps = []
    for b in range(B):
        in_maps.append(
            {
                "img": np.ascontiguousarray(np.asarray(inputs["img_feats"][b], np.float32)),
                "ts": np.ascontiguousarray(np.asarray(inputs["ts_feats"][b], np.float32)),
                "Wq": np.asarray(inputs["Wq"], np.float32),
                "Wk": np.asarray(inputs["Wk"], np.float32),
                "Wv": np.asarray(inputs["Wv"], np.float32),
                "bq": np.asarray(inputs["bq"], np.float32),
                "bk": np.asarray(inputs["bk"], np.float32),
                "bv": np.asarray(inputs["bv"], np.float32),
            }
        )
    res = run_bass_kernel_spmd(nc, in_maps, core_ids=list(range(B)), trace=trace)
    full = np.stack([res.results[b]["out"] for b in range(B)], axis=0)
    return full, res


def kernel(**inputs) -> np.ndarray:
    full, _ = run(inputs, trace=False)
    return full


# revision 5
# speedup vs baseline: 1.6961x; 1.6961x over previous
"""Cross-modal attention Trainium2 kernel (Bass/Tile), data-parallel over batch.

Per core (one batch element):
    q = img @ Wq.T + bq ; k = ts @ Wk.T + bk ; v = ts @ Wv.T + bv
    out = softmax(q @ k.T) @ v

Layout strategy (contraction dim must live on SBUF partitions):
  - img/ts are PE-transposed tile-wise into imgT/tsT [d, n] (fp32r).
  - Weights are PE-transposed into WT [d, e] (fp32r).
  - qT/kT computed as [e, n] (e on partitions) directly; v as [j, e] natural.
  - scores S[i=128, j=2048] accumulated in PSUM as 2x [128,1024] chunk tiles
    (2 banks each, bufs=3) so the next q-tile's score matmuls can start as
    soon as the matching chunk's exp has drained, not after the whole row.
  - row-softmax: per-512 DVE reduce_max (overlaps the score matmuls) + a
    4-wide combine, then 2 chunked ACT Exp(bias=-max, accum_out=partial sums).
  - probs are written as fp16 and transposed via the DMA XBAR into probsT
    [j, i] for the PV matmul (off the PE); 1/sum is applied to the PV result.
  - Projection/score matmuls use fp32r (12-bit significand, 4x faster than
    fp32 on PE); the PV matmul uses fp16 probs/v (11-bit significand).
"""

import numpy as np

import concourse.bass as bass
import concourse.mybir as mybir
import concourse.tile as tile
from concourse import bacc
from concourse.bass_utils import run_bass_kernel_spmd
from concourse.masks import make_identity

B, NQ, NK, D = 8, 2048, 2048, 512
P = 128
DC = D // P        # 4 contraction chunks
EC = D // P        # 4 output-dim chunks
TQ = NQ // P       # 16 query tiles
TK = NK // P       # 16 key tiles
JC = NK // 512     # 4 key chunks of 512 (scores free dim)
IC = NQ // 512     # 4 query chunks of 512 (projection free dim)

F32 = mybir.dt.float32
F32R = mybir.dt.float32r
F16 = mybir.dt.float16
AX = mybir.AxisListType.X
IDENT_FN = mybir.ActivationFunctionType.Identity
EXP_FN = mybir.ActivationFunctionType.Exp


def build():
    nc = bacc.Bacc(None, target_bir_lowering=False)

    img = nc.dram_tensor("img", [NQ, D], F32, kind="ExternalInput")
    ts = nc.dram_tensor("ts", [NK, D], F32, kind="ExternalInput")
    Wq = nc.dram_tensor("Wq", [D, D], F32, kind="ExternalInput")
    Wk = nc.dram_tensor("Wk", [D, D], F32, kind="ExternalInput")
    Wv = nc.dram_tensor("Wv", [D, D], F32, kind="ExternalInput")
    bq = nc.dram_tensor("bq", [D], F32, kind="ExternalInput")
    bk = nc.dram_tensor("bk", [D], F32, kind="ExternalInput")
    bv = nc.dram_tensor("bv", [D], F32, kind="ExternalInput")
    out = nc.dram_tensor("out", [NQ, D], F32, kind="ExternalOutput")

    with tile.TileContext(nc) as tc:
        with (
            tc.tile_pool(name="const", bufs=1) as const_pool,
            tc.tile_pool(name="big", bufs=1) as big,
        ):
            ident = const_pool.tile([P, P], F32)
            make_identity(nc, ident)

            # biases: bq/bk as [P, EC] (e%128 on partitions), bv replicated [P, D].
            # All on the gpsimd SWDGE queue so their tiny descriptors don't sit
            # ahead of the bulk input loads on the HWDGE queue.
            bq_sb = const_pool.tile([P, EC], F32)
            nc.gpsimd.dma_start(bq_sb[:], bq.ap().rearrange("(c p) -> p c", p=P))
            bk_sb = const_pool.tile([P, EC], F32)
            nc.gpsimd.dma_start(bk_sb[:], bk.ap().rearrange("(c p) -> p c", p=P))
            bv_sb = const_pool.tile([P, D], F32)
            nc.gpsimd.dma_start(bv_sb[:], bv.ap().partition_broadcast(P))

            # persistent big operands
            qT = big.tile([P, EC, NQ], F32R)   # qT[p, ec, i] = q[i, ec*128+p]
            kT = big.tile([P, EC, NK], F32R)
            v_sb = big.tile([P, TK, D], F16)   # v_sb[p, jt, e] = v[jt*128+p, e]

            def copy_alt(idx, dst, src):
                """psum->sbuf copy, alternating DVE / ACT to balance engines."""
                if idx % 2 == 0:
                    nc.vector.tensor_copy(dst, src)
                else:
                    nc.scalar.copy(dst, src)

            # ---- Phases 0-3: PE transposes + fp32r projections, pipelined ----
            with (
                tc.tile_pool(name="wt", bufs=1) as wtp,
                tc.tile_pool(name="wstage", bufs=1) as wstage,
                tc.tile_pool(name="xstage", bufs=6) as xstage,
                tc.tile_pool(name="xps", bufs=4, space="PSUM") as xps,
                tc.tile_pool(name="pps", bufs=4, space="PSUM") as pps,
            ):
                WDRAM = {"q": Wq, "k": Wk, "v": Wv}
                w_sbs = {}
                WTs = {}

                def emit_w_dma(wname):
                    w_sb = wstage.tile([P, EC, D], F32, tag=f"wstage_{wname}")
                    # ACT engine's HWDGE queue: weights flow in parallel with
                    # the img tiles on the SP queue.
                    nc.scalar.dma_start(
                        w_sb[:], WDRAM[wname].ap().rearrange("(ec p) d -> p ec d", p=P)
                    )
                    w_sbs[wname] = w_sb

                def emit_w_transpose(wname):
                    w_sb = w_sbs[wname]
                    WT = wtp.tile([P, DC, D], F32R, tag=f"wt_{wname}")
                    for ec in range(EC):
                        for dc in range(DC):
                            pw = xps.tile([P, P], F32, tag="xps")
                            nc.tensor.transpose(
                                pw[:], w_sb[:, ec, dc * P : (dc + 1) * P], ident[:]
                            )
                            copy_alt(
                                ec * DC + dc, WT[:, dc, ec * P : (ec + 1) * P], pw[:]
                            )
                    WTs[wname] = WT

                def emit_xt_chunk(src_dram, xT, ic):
                    """DMA 4 row-tiles of a 512-token chunk, PE-transpose into xT."""
                    for t in range(4):
                        it = 4 * ic + t
                        x_sb = xstage.tile([P, D], F32, tag="xstage")
                        nc.sync.dma_start(
                            x_sb[:], src_dram[it * P : (it + 1) * P, :]
                        )
                        for dc in range(DC):
                            px = xps.tile([P, P], F32, tag="xps")
                            nc.tensor.transpose(
                                px[:], x_sb[:, dc * P : (dc + 1) * P], ident[:]
                            )
                            copy_alt(dc, xT[:, dc, it * P : (it + 1) * P], px[:])

                def emit_proj_chunk(WT, xT, dstT, bias_sb, ic):
                    """dstT[:, :, ic*512:+512] = WT.T @ xT chunk + bias (per-e)."""
                    for ec in range(EC):
                        pq = pps.tile([P, 512], F32, tag="pps")
                        for dc in range(DC):
                            nc.tensor.matmul(
                                pq[:],
                                WT[:, dc, ec * P : (ec + 1) * P],
                                xT[:, dc, ic * 512 : (ic + 1) * 512],
                                start=(dc == 0),
                                stop=(dc == DC - 1),
                            )
                        nc.scalar.activation(
                            out=dstT[:, ec, ic * 512 : (ic + 1) * 512],
                            in_=pq[:],
                            func=IDENT_FN,
                            bias=bias_sb[:, ec : ec + 1],
                            scale=1.0,
                        )

                def emit_v_chunk(tsT, ic):
                    """v rows jt=4ic..4ic+3: v[j,e] = tsT_chunk.T @ WvT + bv."""
                    for t in range(4):
                        jt = 4 * ic + t
                        pv = pps.tile([P, 512], F32, tag="pps")
                        for dc in range(DC):
                            nc.tensor.matmul(
                                pv[:],
                                tsT[:, dc, jt * P : (jt + 1) * P],
                                WTs["v"][:, dc, :],
                                start=(dc == 0),
                                stop=(dc == DC - 1),
                            )
                        nc.vector.tensor_add(v_sb[:, jt, :], pv[:], bv_sb[:])

                with tc.tile_pool(name="xt", bufs=1) as xtp:
                    imgT = xtp.tile([P, DC, NQ], F32R, tag="xT")
                    # DMA issue order sets arrival order: img chunk 0 first
                    # (PE warms up on its transposes), weights in parallel on
                    # the ACT queue.
                    emit_xt_chunk(img, imgT, 0)
                    emit_w_dma("q")
                    emit_w_transpose("q")
                    emit_w_dma("k")
                    emit_xt_chunk(img, imgT, 1)
                    emit_w_transpose("k")
                    emit_w_dma("v")
                    emit_proj_chunk(WTs["q"], imgT, qT, bq_sb, 0)
                    emit_xt_chunk(img, imgT, 2)
                    emit_w_transpose("v")
                    emit_proj_chunk(WTs["q"], imgT, qT, bq_sb, 1)
                    emit_xt_chunk(img, imgT, 3)
                    emit_proj_chunk(WTs["q"], imgT, qT, bq_sb, 2)
                    emit_proj_chunk(WTs["q"], imgT, qT, bq_sb, 3)

                with tc.tile_pool(name="xt2", bufs=1) as xtp2:
                    tsT = xtp2.tile([P, DC, NK], F32R, tag="xT2")
                    for ic in range(IC):
                        emit_xt_chunk(ts, tsT, ic)
                        if ic >= 1:
                            emit_proj_chunk(WTs["k"], tsT, kT, bk_sb, ic - 1)
                            emit_v_chunk(tsT, ic - 1)
                    emit_proj_chunk(WTs["k"], tsT, kT, bk_sb, IC - 1)
                    emit_v_chunk(tsT, IC - 1)

            # ---- Phase 4: attention (software-pipelined by one q-tile) ----
            with (
                tc.tile_pool(name="sps", bufs=3, space="PSUM") as sps,
                tc.tile_pool(name="ops", bufs=2, space="PSUM") as ops,
                tc.tile_pool(name="soft", bufs=3) as soft,
                tc.tile_pool(name="outp", bufs=2) as outp,
            ):
                stash = {}

                def emit_scores_softmax(qt):
                    pmax = soft.tile([P, JC], F32, tag="pmax")
                    chunks = []
                    for jc2 in range(2):
                        Sc = sps.tile([P, 1024], F32, tag="S")
                        chunks.append(Sc)
                        for h in range(2):
                            jc = 2 * jc2 + h
                            for ec in range(EC):
                                nc.tensor.matmul(
                                    Sc[:, h * 512 : (h + 1) * 512],
                                    qT[:, ec, qt * P : (qt + 1) * P],
                                    kT[:, ec, jc * 512 : (jc + 1) * 512],
                                    start=(ec == 0),
                                    stop=(ec == EC - 1),
                                )
                            # chunk max overlaps the next chunk's matmuls
                            nc.vector.reduce_max(
                                pmax[:, jc : jc + 1],
                                Sc[:, h * 512 : (h + 1) * 512],
                                axis=AX,
                            )
                    negmax = soft.tile([P, 1], F32, tag="negmax")
                    nc.vector.reduce_max(negmax[:], pmax[:], axis=AX, negate=True)
                    probs = soft.tile([P, NK], F16, tag="probs")
                    rowsum4 = soft.tile([P, 2], F32, tag="rowsum4")
                    for jc2 in range(2):
                        nc.scalar.activation(
                            out=probs[:, jc2 * 1024 : (jc2 + 1) * 1024],
                            in_=chunks[jc2][:],
                            func=EXP_FN,
                            bias=negmax[:],
                            scale=1.0,
                            accum_out=rowsum4[:, jc2 : jc2 + 1],
                        )
                    # transpose probs via the DMA XBAR (fp16): [i, j] -> [j%P, jt, i]
                    probsT = soft.tile([P, TK, P], F16, tag="probsT")
                    nc.scalar.dma_start_transpose(probsT[:], probs[:])
                    stash[qt] = (probsT, rowsum4)

                def emit_pv(qt):
                    probsT, rowsum4 = stash.pop(qt)
                    rowsum = soft.tile([P, 1], F32, tag="rowsum")
                    nc.vector.reduce_sum(rowsum[:], rowsum4[:], axis=AX)
                    recip = soft.tile([P, 1], F32, tag="recip")
                    nc.vector.reciprocal(recip[:], rowsum[:])
                    po = ops.tile([P, D], F32, tag="po")
                    for jt in range(TK):
                        nc.tensor.matmul(
                            po[:],
                            probsT[:, jt, :],
                            v_sb[:, jt, :],
                            start=(jt == 0),
                            stop=(jt == TK - 1),
                        )
                    o_sb = outp.tile([P, D], F32, tag="o")
                    nc.scalar.mul(out=o_sb[:], in_=po[:], mul=recip[:])
                    nc.sync.dma_start(out[qt * P : (qt + 1) * P, :], o_sb[:])

                for qt in range(TQ):
                    emit_scores_softmax(qt)
                    if qt >= 1:
                        emit_pv(qt - 1)
                emit_pv(TQ - 1)

    nc.compile()
    return nc


_NC_CACHE = None


def _get_nc():
    global _NC_CACHE
    if _NC_CACHE is None:
        _NC_CACHE = build()
    return _NC_CACHE


def run(inputs: dict, trace: bool = False):
    """Run on 8 cores, batch-parallel. Returns (out [B,NQ,D], BassKernelResults)."""
    nc = _get_nc()
    in_maps = []
    for b in range(B):
        in_maps.append(
            {
                "img": np.ascontiguousarray(np.asarray(inputs["img_feats"][b], np.float32)),
                "ts": np.ascontiguousarray(np.asarray(inputs["ts_feats"][b], np.float32)),
                "Wq": np.asarray(inputs["Wq"], np.float32),
                "Wk": np.asarray(inputs["Wk"], np.float32),
                "Wv": np.asarray(inputs["Wv"], np.float32),
                "bq": np.asarray(inputs["bq"], np.float32),
                "bk": np.asarray(inputs["bk"], np.float32),
                "bv": np.asarray(inputs["bv"], np.float32),
            }
        )
    res = run_bass_kernel_spmd(nc, in_maps, core_ids=list(range(B)), trace=trace)
    full = np.stack([res.results[b]["out"] for b in range(B)], axis=0)
    return full, res


def kernel(**inputs) -> np.ndarray:
    full, _ = run(inputs, trace=False)
    return full
